# revision 6
# baseline (speedup 1.0000x reference)
"""GRU-residual trajectory kernel for Trainium2 (8 NeuronCores, data-parallel).

E2 experiment: gates read the f32r staging tile xr (refreshed per step:
ACT copy of h'+ones rows after the h-update, Pool copy of z rows after
the z-update), making all three matmuls 1 cyc/row f32r. State master
stays fp32 in xc; only matmul *inputs* are f32r-rounded. This tests the
f32r-gate-feedback numerics on HW with a minimal delta from the proven
baseline (dz-from-xr at 1.4e-4 HW rel err).
"""

import sys

for p in ("/opt/trn_rl_repo",):
    if p not in sys.path:
        sys.path.insert(0, p)

import numpy as np

import concourse.bacc as bacc
import concourse.bass as bass
import concourse.mybir as mybir
from concourse.tile import TileContext
from concourse.bass_utils import run_bass_kernel_spmd

N_CORES = 8
B_FULL = 16384
BC = B_FULL // N_CORES  # 2048 per core
D = 3
H = 64
K = H + D + 1  # 68 state rows: h (0:64), z (64:67), ones (67)
STEPS = 2048
CHUNK = 512
N_CHUNKS = BC // CHUNK
UNROLL = 16

F32 = mybir.dt.float32
F32R = mybir.dt.float32r
SIG = mybir.ActivationFunctionType.Sigmoid
TANH = mybir.ActivationFunctionType.Tanh
COPY = mybir.ActivationFunctionType.Copy

_NC_CACHE = {}


def _build(steps: int):
    if steps in _NC_CACHE:
        return _NC_CACHE[steps]
    nc = bacc.Bacc(None, target_bir_lowering=False)

    xc0 = nc.dram_tensor("xc0", [K, BC], F32, kind="ExternalInput")
    w1 = nc.dram_tensor("w1", [K, 2 * H], F32R, kind="ExternalInput")
    w23 = nc.dram_tensor("w23", [K, 2 * H], F32R, kind="ExternalInput")
    w5 = nc.dram_tensor("w5", [K, D], F32R, kind="ExternalInput")
    zs = nc.dram_tensor("zs", [steps * D, BC], F32, kind="ExternalOutput")

    with TileContext(nc) as tc:
        with (
            tc.tile_pool(name="state", bufs=1) as state_pool,
            tc.tile_pool(name="wpool", bufs=1) as wpool,
            tc.tile_pool(name="spool", bufs=4) as spool,
            tc.tile_pool(name="tpool", bufs=4) as tpool,
            tc.tile_pool(name="pg1", bufs=2, space="PSUM") as pg1,
            tc.tile_pool(name="phi", bufs=2, space="PSUM") as phi,
            tc.tile_pool(name="pz", bufs=2, space="PSUM") as pz,
        ):
            xc = state_pool.tile([K, BC], F32)
            xr = state_pool.tile([K, BC], F32R, tag="xr")
            w1_t = wpool.tile([K, 2 * H], F32R, tag="w1")
            w23_t = wpool.tile([K, 2 * H], F32R, tag="w23")
            w5_t = wpool.tile([K, D], F32R, tag="w5")

            nc.sync.dma_start(w1_t[:], w1[:])
            nc.sync.dma_start(w23_t[:], w23[:])
            nc.sync.dma_start(w5_t[:], w5[:])
            nc.sync.dma_start(xc[:], xc0[:])  # h=0 | z0 | ones

            # Initial f32r mirror of the full state (rounding producer).
            nc.scalar.activation(xr[:], xc[:], COPY)

            try:
                from concourse.hw_specs import get_activation_tables

                tabs = list(get_activation_tables(nc.m.arch).items())
                need = {SIG, TANH, COPY}
                set_id = next(
                    i for i, (_, fns) in enumerate(tabs) if need <= fns
                )
            except Exception:
                set_id = 2  # sigmoid_and_others
            nc.scalar.add_instruction(
                mybir.InstLoadActFuncSet(
                    name=nc.get_next_instruction_name(),
                    ins=[],
                    outs=[],
                    act_func_set_id=set_id,
                )
            )

            unroll = next(u for u in (UNROLL, 8, 4, 2, 1) if steps % u == 0)
            with tc.For_i(0, steps // unroll) as tu:
                for uu in range(unroll):
                    t = tu * unroll + uu
                    for c in range(N_CHUNKS):
                        cs = slice(c * CHUNK, (c + 1) * CHUNK)

                        g1 = pg1.tile([2 * H, CHUNK], F32)
                        hi = phi.tile([2 * H, CHUNK], F32)
                        nc.tensor.matmul(
                            g1[:], w1_t[:], xr[:, cs], start=True, stop=True
                        )
                        nc.tensor.matmul(
                            hi[:], w23_t[:], xr[:, cs], start=True, stop=True
                        )

                        # s = [u' ; r] (u' rows 0:H base 0, r rows H:2H base 64)
                        s = spool.tile([2 * H, CHUNK], F32, tag="s")
                        nc.scalar.activation(s[:], g1[:], SIG)

                        # t1 lives at base partition 64 to match r
                        t1 = tpool.tile([2 * H, CHUNK], F32, tag="t1")
                        nc.vector.tensor_mul(
                            t1[H : 2 * H, :], s[H : 2 * H, :], hi[H : 2 * H, :]
                        )
                        nc.vector.tensor_add(
                            t1[H : 2 * H, :], t1[H : 2 * H, :], hi[0:H, :]
                        )
                        # tanh bridges base 64 -> base 0
                        n_t = tpool.tile([H, CHUNK], F32, tag="n")
                        nc.scalar.activation(n_t[:], t1[H : 2 * H, :], TANH)

                        t3 = tpool.tile([H, CHUNK], F32, tag="t3")
                        # h' = h + u' * (n - h)
                        nc.gpsimd.tensor_sub(t3[:], n_t[:], xc[0:H, cs])
                        nc.gpsimd.tensor_mul(t3[:], t3[:], s[0:H, :])
                        nc.gpsimd.tensor_add(xc[0:H, cs], xc[0:H, cs], t3[:])

                        # refresh f32r mirror of h' (+ ones row 67)
                        nc.scalar.activation(
                            xr[0:H, cs], xc[0:H, cs], COPY
                        )
                        z_p = pz.tile([D, CHUNK], F32)
                        nc.tensor.matmul(
                            z_p[:], w5_t[:], xr[:, cs], start=True, stop=True
                        )
                        nc.vector.tensor_add(
                            xc[H : H + D, cs], xc[H : H + D, cs], z_p[:]
                        )
                        # refresh f32r mirror of z' for next step's gates
                        nc.gpsimd.tensor_copy(
                            out=xr[H : H + D, cs], in_=xc[H : H + D, cs]
                        )
                        nc.sync.dma_start(
                            zs[bass.ds(t * D, D), cs], xc[H : H + D, cs]
                        )

    nc.finalize()
    _NC_CACHE[steps] = nc
    return nc


def _pack_weights(dt, W_ih, W_hh, b_ih, b_hh, W_head, b_head):
    """Host-side packing of the fused stationary weight matrices."""
    W_ih = np.asarray(W_ih, np.float32)
    W_hh = np.asarray(W_hh, np.float32)
    b_ih = np.asarray(b_ih, np.float32)
    b_hh = np.asarray(b_hh, np.float32)
    W_head = np.asarray(W_head, np.float32)
    b_head = np.asarray(b_head, np.float32)
    dt = np.float32(dt)

    ZR = slice(H, H + D)  # z rows 64:67
    ONE = K - 1  # ones row 67

    w1 = np.zeros((K, 2 * H), np.float32)
    # u gate, negated -> cols 0:H gives sigmoid(-a_u) = 1-u = u'
    w1[0:H, 0:H] = -W_hh[H : 2 * H].T
    w1[ZR, 0:H] = -W_ih[H : 2 * H].T
    w1[ONE, 0:H] = -(b_ih[H : 2 * H] + b_hh[H : 2 * H])
    # r gate -> cols H:2H
    w1[0:H, H : 2 * H] = W_hh[0:H].T
    w1[ZR, H : 2 * H] = W_ih[0:H].T
    w1[ONE, H : 2 * H] = b_ih[0:H] + b_hh[0:H]

    w23 = np.zeros((K, 2 * H), np.float32)
    # i_n -> cols 0:H (z + bias only)
    w23[ZR, 0:H] = W_ih[2 * H : 3 * H].T
    w23[ONE, 0:H] = b_ih[2 * H : 3 * H]
    # h_n -> cols H:2H (h + bias only)
    w23[0:H, H : 2 * H] = W_hh[2 * H : 3 * H].T
    w23[ONE, H : 2 * H] = b_hh[2 * H : 3 * H]

    # w5 computes only dz; exact z accumulates via DVE add in fp32 SBUF
    w5 = np.zeros((K, D), np.float32)
    w5[0:H, :] = dt * W_head.T
    w5[ONE, :] = dt * b_head
    return w1, w23, w5


def kernel(z0, dt, steps, W_ih, W_hh, b_ih, b_hh, W_head, b_head):
    z0 = np.asarray(z0, np.float32)
    steps = int(steps)
    B, d = z0.shape
    assert (B, d) == (B_FULL, D)
    w1, w23, w5 = _pack_weights(dt, W_ih, W_hh, b_ih, b_hh, W_head, b_head)

    nc = _build(steps)
    in_maps = []
    for c in range(N_CORES):
        z0c = z0[c * BC : (c + 1) * BC]  # [BC, 3]
        xc0 = np.zeros((K, BC), np.float32)
        xc0[H : H + D, :] = z0c.T
        xc0[K - 1, :] = 1.0
        in_maps.append({"xc0": xc0, "w1": w1, "w23": w23, "w5": w5})
    res = run_bass_kernel_spmd(nc, in_maps, core_ids=list(range(N_CORES)))

    outs = []
    for c in range(N_CORES):
        zs = res.results[c]["zs"].reshape(steps, D, BC)
        traj = np.empty((BC, steps + 1, D), np.float32)
        traj[:, 0, :] = z0[c * BC : (c + 1) * BC]
        traj[:, 1:, :] = zs.transpose(2, 0, 1)
        outs.append(traj)
    return np.concatenate(outs, axis=0)


# revision 8
# speedup vs baseline: 1.1263x; 1.1263x over previous
"""GRU-residual trajectory kernel for Trainium2 (8 cores, data-parallel).

Per core: batch shard 2048 as 4 chunks of 512 columns, software-pipelined
(chunks phased ~1/4 step apart; per-(chunk,stage) slot-ordered emission).

State representation per chunk:
- h master: PSUM bank (exact fp32), updated only by PE identity-matmul
  accumulation (h += I @ t3), one private bank per chunk (PSUM deps are
  bank-granular; outputs only at partition base 0 pass the ISA check).
- z master: SBUF fp32, updated by DVE add from a dz staged through the
  recycled hi-bank ring.
- XR mirror (f32r, what all matmuls consume at 1 cyc/col): ping-pong pair
  per chunk; h rows 0:64, ones row 64, z rows 96:99 (legal AP bases; pad
  rows carry zero stationary weights). The h-mirror self-recurses in f32r
  (xr' = xr + t3, Pool) and resyncs from the exact PSUM master once per
  16-step unroll to bound drift; the z-mirror is copied from the SBUF
  master each step (Pool, SBUF-only: GPSIMD cannot touch PSUM on HW).
- Lazy-z weight fold: gate blocks use (A_h + dt*A_z@W_head) on h_t and
  A_z on z_{t-1}, so gates may read a one-step-stale z mirror and the
  whole z path drops off the loop-carried critical chain. The mirror
  seeds with zeta_0 = z0 - dt*b_head; the PSUM-free z master seeds with
  exact z0. The z mirror written in step t holds z_t, which is exactly
  trajectory row t: the kernel emits the full trajectory incl. the z0
  row ([steps+1]*D rows, f32r ~12-bit rounding on the output only).

Per step, per chunk (engine in parentheses; b = the chunk's PSUM h bank):
  g1 = W1.T @ XR    -> bankG [u';r]      (PE, f32r)
  XRn.z = z_sb                           (Pool)   -> DMA row t
  S  = sigmoid(bankG)                    (ACT)
  Q  = S[r] * bankN[h_n]  (f32r)         (DVE: PSUM-capable)
  bankN[0:64] += I @ Q    (t1)           (PE)
  N  = tanh(bankN[0:64])                 (ACT)
  V  = N - XR.h                          (Pool, SBUF-only)
  T3 = S[u'] * V          (f32r)         (Pool)
  b += I @ T3             (exact h')     (PE)
  XRn.h = XR.h + T3  | resync f32r(b)    (Pool | DVE once/16)
  dzb = W5.T @ XRn[h,1]                  (PE, via hi-bank ring)
  z_sb += dzb                            (DVE)

Measured: rel err 3.03e-04 on HW (2048 steps); CoreSim steady state
~8.7us/step vs ~9.5us/step for the previous staged-f32r baseline.
Precision ledger (HW): baseline staged-f32r 1.41e-4; f32r gates from a
fresh mirror 1.26e-4; this kernel (mirror recurrence + lazy-z) 3.03e-4;
tolerance 2e-2.
"""

import sys

for p in ("/opt/trn_rl_repo",):
    if p not in sys.path:
        sys.path.insert(0, p)

import numpy as np

import concourse.bacc as bacc
import concourse.bass as bass
import concourse.mybir as mybir
from concourse.tile import TileContext
from concourse.bass_utils import run_bass_kernel_spmd

N_CORES = 8
B_FULL = 16384
BC = B_FULL // N_CORES
D = 3
H = 64
# XR rows: h 0:64, ones 64, pad, z 96:99 (z needs a legal partition base:
# AP starts must be 0/32/64/96; pad rows have zero stationary weights)
K = 100
HO = H + 1  # h rows + ones row (dz matmul moving slice)
Z0 = 96
CHUNK = 512
N_CHUNKS = BC // CHUNK
UNROLL = 16

F32 = mybir.dt.float32
F32R = mybir.dt.float32r
SIG = mybir.ActivationFunctionType.Sigmoid
TANH = mybir.ActivationFunctionType.Tanh
COPY = mybir.ActivationFunctionType.Copy

_NC_CACHE = {}


def _build(steps: int, sigma=None):
    key = (steps, None if sigma is None else tuple(sorted(sigma.items())))
    if key in _NC_CACHE:
        return _NC_CACHE[key]
    nc = bacc.Bacc(None, target_bir_lowering=False)
    nc._emit_map = {}  # inst name -> (c, k) for schedule iteration

    xr0 = nc.dram_tensor("xr0", [K, BC], F32R, kind="ExternalInput")
    w1 = nc.dram_tensor("w1", [K, 2 * H], F32R, kind="ExternalInput")
    w23 = nc.dram_tensor("w23", [K, 2 * H], F32R, kind="ExternalInput")
    w5 = nc.dram_tensor("w5", [HO, D], F32R, kind="ExternalInput")
    i1 = nc.dram_tensor("i1", [H, 2 * H], F32R, kind="ExternalInput")
    i2 = nc.dram_tensor("i2", [H, H], F32R, kind="ExternalInput")
    zs3 = nc.dram_tensor("zs3", [K, 2 * H], F32R, kind="ExternalInput")
    wz0 = nc.dram_tensor("wz0", [K, 2 * H], F32R, kind="ExternalInput")
    bhd = nc.dram_tensor("bhd", [D, 1], F32, kind="ExternalInput")
    zs = nc.dram_tensor("zs", [(steps + 1) * D, BC], F32R, kind="ExternalOutput")

    with TileContext(nc) as tc:
        with (
            tc.tile_pool(name="state", bufs=1) as state_pool,
            tc.tile_pool(name="wpool", bufs=1) as wpool,
            tc.tile_pool(name="spool", bufs=4) as spool,
            tc.tile_pool(name="tpool", bufs=4) as tpool,
            tc.tile_pool(name="pst", bufs=1, space="PSUM") as pst,
            tc.tile_pool(name="pg1", bufs=2, space="PSUM") as pg1,
            tc.tile_pool(name="phi", bufs=2, space="PSUM") as phi,
        ):
            # ping-pong mirrors: gates/dz of step t read xrs[c][t%2];
            # mirrors written during step t go to xrs[c][(t+1)%2]. This keeps
            # the early z-mirror write off the h-mirror's false WAW chain
            # (SBUF subtile deps are byte-range granular, partition-blind).
            xrs = [
                [
                    state_pool.tile(
                        [K, CHUNK], F32R, tag=f"xr{c}_{p}", name=f"xr{c}_{p}"
                    )
                    for p in range(2)
                ]
                for c in range(N_CHUNKS)
            ]
            w1_t = wpool.tile([K, 2 * H], F32R, tag="w1")
            w23_t = wpool.tile([K, 2 * H], F32R, tag="w23")
            w5_t = wpool.tile([HO, D], F32R, tag="w5")
            i1_t = wpool.tile([H, 2 * H], F32R, tag="i1")
            i2_t = wpool.tile([H, H], F32R, tag="i2")
            zs3_t = wpool.tile([K, 2 * H], F32R, tag="zs3")
            wz0_t = wpool.tile([K, 2 * H], F32R, tag="wz0")
            bhd_t = wpool.tile([D, 1], F32, tag="bhd")

            nc.sync.dma_start(w1_t[:], w1[:])
            nc.sync.dma_start(w23_t[:], w23[:])
            nc.sync.dma_start(w5_t[:], w5[:])
            nc.sync.dma_start(i1_t[:], i1[:])
            nc.sync.dma_start(i2_t[:], i2[:])
            nc.sync.dma_start(zs3_t[:], zs3[:])
            nc.sync.dma_start(wz0_t[:], wz0[:])
            nc.sync.dma_start(bhd_t[:], bhd[:])
            for c in range(N_CHUNKS):
                for p in range(2):
                    nc.sync.dma_start(
                        xrs[c][p][:], xr0[:, c * CHUNK : (c + 1) * CHUNK]
                    )

            try:
                from concourse.hw_specs import get_activation_tables

                tabs = list(get_activation_tables(nc.m.arch).items())
                need = {SIG, TANH, COPY}
                set_id = next(
                    i for i, (_, fns) in enumerate(tabs) if need <= fns
                )
            except Exception:
                set_id = 2
            nc.scalar.add_instruction(
                mybir.InstLoadActFuncSet(
                    name=nc.get_next_instruction_name(),
                    ins=[],
                    outs=[],
                    act_func_set_id=set_id,
                )
            )

            # Persistent PSUM h-state: one private bank per chunk, matmul
            # outputs at partition base 0 only (ISA constraint). z masters
            # live in SBUF fp32, updated by DVE from a dz staged through the
            # recycled hi-bank ring.
            sts = [
                pst.tile([H, CHUNK], F32, tag=f"st{c}", name=f"st{c}")
                for c in range(N_CHUNKS)
            ]
            zsb = [
                state_pool.tile([D, CHUNK], F32, tag=f"zsb{c}", name=f"zsb{c}")
                for c in range(N_CHUNKS)
            ]

            def h_slice(c):
                return sts[c][:]

            for c in range(N_CHUNKS):
                # h0 = 0 via explicit zero matmul (marks + clears pending)
                nc.tensor.matmul(
                    sts[c][:], zs3_t[:, 0:H], xrs[c][0][:],
                    start=True, stop=True, skip_group_check=True,
                )
                # z0 (undo the lazy offset): z_sb = I3*xr.z + dt*b_head
                # computed on DVE from the mirror, exact in fp32
                nc.vector.tensor_scalar_mul(
                    zsb[c][:], xrs[c][0][Z0 : Z0 + D, :], 1.0
                )
                nc.vector.tensor_scalar_add(
                    zsb[c][:], zsb[c][:], bhd_t[0:D, 0:1]
                )

            unroll = next(u for u in (UNROLL, 8, 4, 2, 1) if steps % u == 0)
            CS = [slice(c * CHUNK, (c + 1) * CHUNK) for c in range(N_CHUNKS)]
            with tc.For_i(0, steps // unroll) as tu:
                # Software pipeline: chunk c runs 3 stages behind chunk c-1,
                # so at most ~2 g1/hi PSUM banks are live at once (8-bank
                # budget: 3 state + 2 g1 + 3 hi). Emission order = slot order.
                live = {}

                def s0(c, t):
                    g1 = pg1.tile([2 * H, CHUNK], F32, tag="g1", name="g1")
                    hi = phi.tile([2 * H, CHUNK], F32, tag="hi", name="hi")
                    live[c] = (g1, hi)
                    nc.tensor.matmul(
                        g1[:], w1_t[:], xrs[c][t % 2][:], start=True, stop=True
                    )
                    nc.tensor.matmul(
                        hi[:], w23_t[:], xrs[c][t % 2][:], start=True, stop=True
                    )

                def s1(c, t):
                    g1, hi = live[c]
                    s = spool.tile(
                        [2 * H, CHUNK], F32, tag=f"s{c}", name=f"s{c}"
                    )
                    live[c] = (g1, hi, s)
                    nc.scalar.activation(s[:], g1[:], SIG)

                def s2(c, t):
                    # z mirror from the SBUF master (Pool-legal), pre z-add
                    nc.gpsimd.tensor_copy(
                        out=xrs[c][(t + 1) % 2][Z0 : Z0 + D, :],
                        in_=zsb[c][:],
                    )

                def s3(c, t):
                    nc.sync.dma_start(
                        zs[bass.ds(t * D, D), CS[c]],
                        xrs[c][(t + 1) % 2][Z0 : Z0 + D, :],
                    )

                def s4(c, t):
                    # q = r * h_n (DVE: reads PSUM)
                    g1, hi, s = live[c]
                    q = tpool.tile([H, CHUNK], F32R, tag=f"q{c}", name=f"q{c}")
                    live[c] = (hi, s, q)
                    nc.vector.tensor_mul(
                        q[:], s[H : 2 * H, :], hi[H : 2 * H, :]
                    )

                def s5(c, t):
                    hi, s, q = live[c]
                    nc.tensor.matmul(
                        hi[:], i1_t[:], q[:],
                        start=False, stop=True, skip_group_check=True,
                    )

                def s6(c, t):
                    hi, s, q = live[c]
                    n = tpool.tile([H, CHUNK], F32, tag=f"n{c}", name=f"n{c}")
                    live[c] = (s, n)
                    nc.scalar.activation(n[:], hi[0:H, :], TANH)

                def s7(c, t):
                    # v = n - h_mirror (Pool, all-SBUF)
                    s, n = live[c]
                    v = tpool.tile([H, CHUNK], F32, tag=f"v{c}", name=f"v{c}")
                    live[c] = (s, v)
                    nc.gpsimd.tensor_sub(v[:], n[:], xrs[c][t % 2][0:H, :])

                def s8(c, t):
                    # t3 = u' * v (Pool, f32r out)
                    s, v = live[c]
                    t3 = tpool.tile(
                        [H, CHUNK], F32R, tag=f"t3{c}", name=f"t3{c}"
                    )
                    live[c] = (t3,)
                    nc.gpsimd.tensor_mul(t3[:], s[0:H, :], v[:])

                def s9(c, t):
                    # exact master: h += I @ t3 in PSUM
                    (t3,) = live[c]
                    nc.tensor.matmul(
                        h_slice(c), i2_t[:], t3[:],
                        start=False, stop=False, skip_group_check=True,
                    )

                def s10(c, t, uu=None):
                    # mirror update: recurrent f32r add (Pool) most steps;
                    # resync from the exact PSUM master (DVE) once per
                    # unrolled body to bound mirror drift
                    (t3,) = live[c]
                    if uu == unroll - 1:
                        nc.vector.tensor_copy(
                            out=xrs[c][(t + 1) % 2][0:H, :], in_=h_slice(c)
                        )
                    else:
                        nc.gpsimd.tensor_add(
                            xrs[c][(t + 1) % 2][0:H, :],
                            xrs[c][t % 2][0:H, :],
                            t3[:],
                        )

                def s11(c, t):
                    dzb = phi.tile(
                        [2 * H, CHUNK], F32, tag="hi", name="dzb"
                    )
                    live[c] = live[c] + (dzb,)
                    nc.tensor.matmul(
                        dzb[0:D, :], w5_t[:], xrs[c][(t + 1) % 2][0:HO, :],
                        start=True, stop=True,
                    )

                def s12(c, t):
                    tup = live[c]
                    dzb = tup[-1]
                    nc.vector.tensor_add(zsb[c][:], zsb[c][:], dzb[0:D, :])

                stages = [
                    s0, s2, s1, s3, s4, s5, s6, s7, s8, s9, s10, s11, s12
                ]
                import os as _os
                dur = [426, 527, 712, 80, 758, 313, 712, 527, 527, 313, 527, 313, 758]
                pref = [0]
                for dd in dur:
                    pref.append(pref[-1] + dd)
                P = pref[-1]
                phase = P // N_CHUNKS
                sig_of = {
                    (c, k): c * phase + pref[k]
                    for c in range(N_CHUNKS)
                    for k in range(13)
                }
                events = sorted(
                    (t * P + sig_of[(c, k)], k, c, t)
                    for t in range(unroll)
                    for c in range(N_CHUNKS)
                    for k in range(13)
                )
                for _, k, c, t in events:
                    g = tu * unroll + t
                    if stages[k] is s10:
                        s10(c, g, uu=t)
                    else:
                        stages[k](c, g)

            # tail: emit z_steps (trajectory row `steps`)
            for c in range(N_CHUNKS):
                nc.vector.tensor_copy(
                    out=xrs[c][steps % 2][Z0 : Z0 + D, :], in_=zsb[c][:]
                )
                nc.sync.dma_start(
                    zs[bass.ds(steps * D, D), CS[c]],
                    xrs[c][steps % 2][Z0 : Z0 + D, :]
                )

    nc.finalize()
    _NC_CACHE[key] = nc
    return nc


def _pack_weights(dt, W_ih, W_hh, b_ih, b_hh, W_head, b_head):
    """Lazy-z fold: each gate block (A_h, A_z, a0) with preact
    A_h h_t + A_z z_t + a0 becomes (A_h + dt A_z W_head) h_t
    + A_z z_{t-1} + (a0 + dt A_z b_head)."""
    W_ih = np.asarray(W_ih, np.float64)
    W_hh = np.asarray(W_hh, np.float64)
    b_ih = np.asarray(b_ih, np.float64)
    b_hh = np.asarray(b_hh, np.float64)
    W_head = np.asarray(W_head, np.float64)
    b_head = np.asarray(b_head, np.float64)
    dt = float(dt)

    ONE = H
    ZR = slice(96, 96 + D)

    def fold(A_h, A_z, a0):
        return (
            A_h + dt * (A_z @ W_head),
            A_z,
            a0 + dt * (A_z @ b_head),
        )

    # gate blocks in PyTorch order: r = rows 0:H, u = H:2H, n = 2H:3H
    r_h, r_z, r_0 = fold(W_hh[0:H], W_ih[0:H], b_ih[0:H] + b_hh[0:H])
    u_h, u_z, u_0 = fold(
        W_hh[H : 2 * H], W_ih[H : 2 * H], b_ih[H : 2 * H] + b_hh[H : 2 * H]
    )
    # i_n depends only on z (A_h = 0); h_n only on h (A_z = 0, no fold)
    in_h, in_z, in_0 = fold(np.zeros((H, H)), W_ih[2 * H : 3 * H], b_ih[2 * H :])
    hn_h, hn_0 = W_hh[2 * H : 3 * H], b_hh[2 * H : 3 * H]

    w1 = np.zeros((K, 2 * H))
    # u' block (negated): sigmoid(-a_u) = 1-u
    w1[0:H, 0:H] = -u_h.T
    w1[ZR, 0:H] = -u_z.T
    w1[ONE, 0:H] = -u_0
    # r block
    w1[0:H, H : 2 * H] = r_h.T
    w1[ZR, H : 2 * H] = r_z.T
    w1[ONE, H : 2 * H] = r_0

    w23 = np.zeros((K, 2 * H))
    # i_n block (h-part nonzero after fold!)
    w23[0:H, 0:H] = in_h.T
    w23[ZR, 0:H] = in_z.T
    w23[ONE, 0:H] = in_0
    # h_n block
    w23[0:H, H : 2 * H] = hn_h.T
    w23[ONE, H : 2 * H] = hn_0

    w5 = np.zeros((H + 1, D))
    w5[0:H, :] = dt * W_head.T
    w5[H, :] = dt * b_head

    i1 = np.zeros((H, 2 * H))
    i1[0:H, 0:H] = np.eye(H)
    i2 = np.eye(H)

    # h-seed stationary: all zeros (h0 = 0); z seeding is done on DVE
    zs3 = np.zeros((K, 2 * H))
    bhd = (dt * b_head).reshape(D, 1)

    wz0 = np.zeros((K, 2 * H))

    f32 = np.float32
    return tuple(
        a.astype(f32) for a in (w1, w23, w5, i1, i2, zs3, wz0, bhd)
    ) + (f32(dt), b_head.astype(f32))


def kernel(z0, dt, steps, W_ih, W_hh, b_ih, b_hh, W_head, b_head):
    z0 = np.asarray(z0, np.float32)
    steps = int(steps)
    B, d = z0.shape
    assert (B, d) == (B_FULL, D)
    w1, w23, w5, i1, i2, zs3, wz0, bhd, dtf, bh_f = _pack_weights(
        dt, W_ih, W_hh, b_ih, b_hh, W_head, b_head
    )

    nc = _build(steps)
    in_maps = []
    for c in range(N_CORES):
        z0c = z0[c * BC : (c + 1) * BC]
        xr0 = np.zeros((K, BC), np.float32)
        # lazy-z: mirror seeds with zeta_0 = z0 - dt*b_head
        xr0[96 : 96 + D, :] = (z0c - dtf * bh_f[None, :]).T
        xr0[H, :] = 1.0
        in_maps.append(
            {
                "xr0": xr0,
                "w1": w1,
                "w23": w23,
                "w5": w5,
                "i1": i1,
                "i2": i2,
                "zs3": zs3,
                "wz0": wz0,
                "bhd": bhd,
            }
        )
    res = run_bass_kernel_spmd(nc, in_maps, core_ids=list(range(N_CORES)))

    outs = []
    for c in range(N_CORES):
        zsv = np.asarray(res.results[c]["zs"], np.float32)
        traj = (
            zsv.reshape(steps + 1, D, BC).transpose(2, 0, 1).astype(np.float32)
        )
        outs.append(traj)
    return np.concatenate(outs, axis=0)


# revision 9
# speedup vs baseline: 1.1352x; 1.0079x over previous
"""GRU-residual trajectory kernel for Trainium2 (8 cores, data-parallel).

Per core: batch shard 2048 as 4 chunks of 512 columns, software-pipelined
(chunks phased ~1/4 step apart; per-(chunk,stage) slot-ordered emission).

State representation per chunk:
- h master: PSUM bank (exact fp32), updated only by PE identity-matmul
  accumulation (h += I @ t3), one private bank per chunk (PSUM deps are
  bank-granular; outputs only at partition base 0 pass the ISA check).
- z master: SBUF fp32, updated by DVE add from a dz staged through the
  recycled hi-bank ring.
- XR mirror (f32r, what all matmuls consume at 1 cyc/col): ping-pong pair
  per chunk; h rows 0:64, ones row 64, z rows 96:99 (legal AP bases; pad
  rows carry zero stationary weights). The h-mirror self-recurses in f32r
  (xr' = xr + t3, Pool) and resyncs from the exact PSUM master once per
  16-step unroll to bound drift; the z-mirror is copied from the SBUF
  master each step (Pool, SBUF-only: GPSIMD cannot touch PSUM on HW).
- Lazy-z weight fold: gate blocks use (A_h + dt*A_z@W_head) on h_t and
  A_z on z_{t-1}, so gates may read a one-step-stale z mirror and the
  whole z path drops off the loop-carried critical chain. The mirror
  seeds with zeta_0 = z0 - dt*b_head; the PSUM-free z master seeds with
  exact z0. The z mirror written in step t holds z_t, which is exactly
  trajectory row t: the kernel emits the full trajectory incl. the z0
  row ([steps+1]*D rows, f32r ~12-bit rounding on the output only).

Per step, per chunk (engine in parentheses; b = the chunk's PSUM h bank):
  g1 = W1.T @ XR    -> bankG [u';r]      (PE, f32r)
  XRn.z = z_sb                           (Pool)   -> DMA row t
  S  = sigmoid(bankG)                    (ACT)
  Q  = S[r] * bankN[h_n]  (f32r)         (DVE: PSUM-capable)
  bankN[0:64] += I @ Q    (t1)           (PE)
  N  = tanh(bankN[0:64])                 (ACT)
  V  = N - XR.h                          (Pool, SBUF-only)
  T3 = S[u'] * V          (f32r)         (Pool)
  b += I @ T3             (exact h')     (PE)
  XRn.h = XR.h + T3  | resync f32r(b)    (Pool | DVE once/16)
  dzb = W5.T @ XRn[h,1]                  (PE, via hi-bank ring)
  z_sb += dzb                            (DVE)

Measured: rel err 3.03e-04 on HW (2048 steps); CoreSim steady state
~8.7us/step vs ~9.5us/step for the previous staged-f32r baseline.
Precision ledger (HW): baseline staged-f32r 1.41e-4; f32r gates from a
fresh mirror 1.26e-4; this kernel (mirror recurrence + lazy-z) 3.03e-4;
tolerance 2e-2.
"""

import sys

for p in ("/opt/trn_rl_repo",):
    if p not in sys.path:
        sys.path.insert(0, p)

import numpy as np

import concourse.bacc as bacc
import concourse.bass as bass
import concourse.mybir as mybir
from concourse.tile import TileContext
from concourse.bass_utils import run_bass_kernel_spmd

N_CORES = 8
B_FULL = 16384
BC = B_FULL // N_CORES
D = 3
H = 64
# XR rows: h 0:64, ones 64, pad, z 96:99 (z needs a legal partition base:
# AP starts must be 0/32/64/96; pad rows have zero stationary weights)
K = 100
HO = H + 1  # h rows + ones row (dz matmul moving slice)
Z0 = 96
CHUNK = 512
N_CHUNKS = BC // CHUNK
UNROLL = 16

F32 = mybir.dt.float32
F32R = mybir.dt.float32r
SIG = mybir.ActivationFunctionType.Sigmoid
TANH = mybir.ActivationFunctionType.Tanh
COPY = mybir.ActivationFunctionType.Copy

_NC_CACHE = {}


def _build(steps: int, sigma=None):
    key = (steps, None if sigma is None else tuple(sorted(sigma.items())))
    if key in _NC_CACHE:
        return _NC_CACHE[key]
    nc = bacc.Bacc(None, target_bir_lowering=False)
    nc._emit_map = {}  # inst name -> (c, k) for schedule iteration

    xr0 = nc.dram_tensor("xr0", [K, BC], F32R, kind="ExternalInput")
    w1 = nc.dram_tensor("w1", [K, 2 * H], F32R, kind="ExternalInput")
    w23 = nc.dram_tensor("w23", [K, 2 * H], F32R, kind="ExternalInput")
    w5 = nc.dram_tensor("w5", [HO, D], F32R, kind="ExternalInput")
    i1 = nc.dram_tensor("i1", [H, 2 * H], F32R, kind="ExternalInput")
    i2 = nc.dram_tensor("i2", [H, H], F32R, kind="ExternalInput")
    zs3 = nc.dram_tensor("zs3", [K, 2 * H], F32R, kind="ExternalInput")
    wz0 = nc.dram_tensor("wz0", [K, 2 * H], F32R, kind="ExternalInput")
    bhd = nc.dram_tensor("bhd", [D, 1], F32, kind="ExternalInput")
    zs = nc.dram_tensor("zs", [(steps + 1) * D, BC], F32R, kind="ExternalOutput")

    with TileContext(nc) as tc:
        with (
            tc.tile_pool(name="state", bufs=1) as state_pool,
            tc.tile_pool(name="wpool", bufs=1) as wpool,
            tc.tile_pool(name="spool", bufs=4) as spool,
            tc.tile_pool(name="tpool", bufs=4) as tpool,
            tc.tile_pool(name="pst", bufs=1, space="PSUM") as pst,
            tc.tile_pool(name="pg1", bufs=2, space="PSUM") as pg1,
            tc.tile_pool(name="phi", bufs=2, space="PSUM") as phi,
        ):
            # ping-pong mirrors: gates/dz of step t read xrs[c][t%2];
            # mirrors written during step t go to xrs[c][(t+1)%2]. This keeps
            # the early z-mirror write off the h-mirror's false WAW chain
            # (SBUF subtile deps are byte-range granular, partition-blind).
            xrs = [
                [
                    state_pool.tile(
                        [K, CHUNK], F32R, tag=f"xr{c}_{p}", name=f"xr{c}_{p}"
                    )
                    for p in range(2)
                ]
                for c in range(N_CHUNKS)
            ]
            w1_t = wpool.tile([K, 2 * H], F32R, tag="w1")
            w23_t = wpool.tile([K, 2 * H], F32R, tag="w23")
            w5_t = wpool.tile([HO, D], F32R, tag="w5")
            i1_t = wpool.tile([H, 2 * H], F32R, tag="i1")
            i2_t = wpool.tile([H, H], F32R, tag="i2")
            zs3_t = wpool.tile([K, 2 * H], F32R, tag="zs3")
            wz0_t = wpool.tile([K, 2 * H], F32R, tag="wz0")
            bhd_t = wpool.tile([D, 1], F32, tag="bhd")

            nc.sync.dma_start(w1_t[:], w1[:])
            nc.sync.dma_start(w23_t[:], w23[:])
            nc.sync.dma_start(w5_t[:], w5[:])
            nc.sync.dma_start(i1_t[:], i1[:])
            nc.sync.dma_start(i2_t[:], i2[:])
            nc.sync.dma_start(zs3_t[:], zs3[:])
            nc.sync.dma_start(wz0_t[:], wz0[:])
            nc.sync.dma_start(bhd_t[:], bhd[:])
            for c in range(N_CHUNKS):
                for p in range(2):
                    nc.sync.dma_start(
                        xrs[c][p][:], xr0[:, c * CHUNK : (c + 1) * CHUNK]
                    )

            try:
                from concourse.hw_specs import get_activation_tables

                tabs = list(get_activation_tables(nc.m.arch).items())
                need = {SIG, TANH, COPY}
                set_id = next(
                    i for i, (_, fns) in enumerate(tabs) if need <= fns
                )
            except Exception:
                set_id = 2
            nc.scalar.add_instruction(
                mybir.InstLoadActFuncSet(
                    name=nc.get_next_instruction_name(),
                    ins=[],
                    outs=[],
                    act_func_set_id=set_id,
                )
            )

            # Persistent PSUM h-state: one private bank per chunk, matmul
            # outputs at partition base 0 only (ISA constraint). z masters
            # live in SBUF fp32, updated by DVE from a dz staged through the
            # recycled hi-bank ring.
            sts = [
                pst.tile([H, CHUNK], F32, tag=f"st{c}", name=f"st{c}")
                for c in range(N_CHUNKS)
            ]
            zsb = [
                state_pool.tile([D, CHUNK], F32, tag=f"zsb{c}", name=f"zsb{c}")
                for c in range(N_CHUNKS)
            ]

            def h_slice(c):
                return sts[c][:]

            for c in range(N_CHUNKS):
                # h0 = 0 via explicit zero matmul (marks + clears pending)
                nc.tensor.matmul(
                    sts[c][:], zs3_t[:, 0:H], xrs[c][0][:],
                    start=True, stop=True, skip_group_check=True,
                )
                # z0 (undo the lazy offset): z_sb = I3*xr.z + dt*b_head
                # computed on DVE from the mirror, exact in fp32
                nc.vector.tensor_scalar_mul(
                    zsb[c][:], xrs[c][0][Z0 : Z0 + D, :], 1.0
                )
                nc.vector.tensor_scalar_add(
                    zsb[c][:], zsb[c][:], bhd_t[0:D, 0:1]
                )

            unroll = next(u for u in (UNROLL, 8, 4, 2, 1) if steps % u == 0)
            CS = [slice(c * CHUNK, (c + 1) * CHUNK) for c in range(N_CHUNKS)]
            with tc.For_i(0, steps // unroll) as tu:
                # Software pipeline: chunk c runs 3 stages behind chunk c-1,
                # so at most ~2 g1/hi PSUM banks are live at once (8-bank
                # budget: 3 state + 2 g1 + 3 hi). Emission order = slot order.
                live = {}

                def s0(c, t):
                    g1 = pg1.tile([2 * H, CHUNK], F32, tag="g1", name="g1")
                    hi = phi.tile([2 * H, CHUNK], F32, tag="hi", name="hi")
                    live[c] = (g1, hi)
                    nc.tensor.matmul(
                        g1[:], w1_t[:], xrs[c][t % 2][:], start=True, stop=True
                    )
                    nc.tensor.matmul(
                        hi[:], w23_t[:], xrs[c][t % 2][:], start=True, stop=True
                    )

                def s1(c, t):
                    g1, hi = live[c]
                    s = spool.tile(
                        [2 * H, CHUNK], F32, tag=f"s{c}", name=f"s{c}"
                    )
                    live[c] = (g1, hi, s)
                    nc.scalar.activation(s[:], g1[:], SIG)

                def s2(c, t):
                    # z mirror from the SBUF master (Pool-legal), pre z-add
                    nc.gpsimd.tensor_copy(
                        out=xrs[c][(t + 1) % 2][Z0 : Z0 + D, :],
                        in_=zsb[c][:],
                    )

                def s3(c, t):
                    nc.sync.dma_start(
                        zs[bass.ds(t * D, D), CS[c]],
                        xrs[c][(t + 1) % 2][Z0 : Z0 + D, :],
                    )

                def s4(c, t):
                    # q = r * h_n (DVE: reads PSUM)
                    g1, hi, s = live[c]
                    q = tpool.tile([H, CHUNK], F32R, tag=f"q{c}", name=f"q{c}")
                    live[c] = (hi, s, q)
                    nc.vector.tensor_mul(
                        q[:], s[H : 2 * H, :], hi[H : 2 * H, :]
                    )

                def s5(c, t):
                    hi, s, q = live[c]
                    nc.tensor.matmul(
                        hi[:], i1_t[:], q[:],
                        start=False, stop=True, skip_group_check=True,
                    )

                def s6(c, t):
                    hi, s, q = live[c]
                    n = tpool.tile([H, CHUNK], F32, tag=f"n{c}", name=f"n{c}")
                    live[c] = (s, n)
                    nc.scalar.activation(n[:], hi[0:H, :], TANH)

                def s7(c, t):
                    # v = n - h_mirror (Pool, all-SBUF)
                    s, n = live[c]
                    v = tpool.tile([H, CHUNK], F32, tag=f"v{c}", name=f"v{c}")
                    live[c] = (s, v)
                    nc.gpsimd.tensor_sub(v[:], n[:], xrs[c][t % 2][0:H, :])

                def s8(c, t):
                    # t3 = u' * v (Pool, f32r out)
                    s, v = live[c]
                    t3 = tpool.tile(
                        [H, CHUNK], F32R, tag=f"t3{c}", name=f"t3{c}"
                    )
                    live[c] = (t3,)
                    nc.gpsimd.tensor_mul(t3[:], s[0:H, :], v[:])

                def s9(c, t):
                    # exact master: h += I @ t3 in PSUM
                    (t3,) = live[c]
                    nc.tensor.matmul(
                        h_slice(c), i2_t[:], t3[:],
                        start=False, stop=False, skip_group_check=True,
                    )

                def s10(c, t, uu=None):
                    # mirror update: recurrent f32r add (Pool) most steps;
                    # resync from the exact PSUM master (DVE) once per
                    # unrolled body to bound mirror drift
                    (t3,) = live[c]
                    if uu == unroll - 1:
                        nc.vector.tensor_copy(
                            out=xrs[c][(t + 1) % 2][0:H, :], in_=h_slice(c)
                        )
                    else:
                        nc.gpsimd.tensor_add(
                            xrs[c][(t + 1) % 2][0:H, :],
                            xrs[c][t % 2][0:H, :],
                            t3[:],
                        )

                def s11(c, t):
                    dzb = phi.tile(
                        [2 * H, CHUNK], F32, tag="hi", name="dzb"
                    )
                    live[c] = live[c] + (dzb,)
                    nc.tensor.matmul(
                        dzb[0:D, :], w5_t[:], xrs[c][(t + 1) % 2][0:HO, :],
                        start=True, stop=True,
                    )

                def s12(c, t):
                    tup = live[c]
                    dzb = tup[-1]
                    nc.vector.tensor_add(zsb[c][:], zsb[c][:], dzb[0:D, :])

                stages = [
                    s0, s1, s4, s5, s6, s7, s8, s9, s10, s2, s3, s11, s12
                ]
                import os as _os
                dur = [426, 712, 758, 313, 712, 527, 527, 313, 527, 427, 80, 313, 758]
                pref = [0]
                for dd in dur:
                    pref.append(pref[-1] + dd)
                P = pref[-1]
                phase = P // N_CHUNKS
                sig_of = {
                    (c, k): c * phase + pref[k]
                    for c in range(N_CHUNKS)
                    for k in range(13)
                }
                events = sorted(
                    (t * P + sig_of[(c, k)], k, c, t)
                    for t in range(unroll)
                    for c in range(N_CHUNKS)
                    for k in range(13)
                )
                for _, k, c, t in events:
                    g = tu * unroll + t
                    if stages[k] is s10:
                        s10(c, g, uu=t)
                    else:
                        stages[k](c, g)

            # tail: emit z_steps (trajectory row `steps`)
            for c in range(N_CHUNKS):
                nc.vector.tensor_copy(
                    out=xrs[c][steps % 2][Z0 : Z0 + D, :], in_=zsb[c][:]
                )
                nc.sync.dma_start(
                    zs[bass.ds(steps * D, D), CS[c]],
                    xrs[c][steps % 2][Z0 : Z0 + D, :]
                )

    nc.finalize()
    _NC_CACHE[key] = nc
    return nc


def _pack_weights(dt, W_ih, W_hh, b_ih, b_hh, W_head, b_head):
    """Lazy-z fold: each gate block (A_h, A_z, a0) with preact
    A_h h_t + A_z z_t + a0 becomes (A_h + dt A_z W_head) h_t
    + A_z z_{t-1} + (a0 + dt A_z b_head)."""
    W_ih = np.asarray(W_ih, np.float64)
    W_hh = np.asarray(W_hh, np.float64)
    b_ih = np.asarray(b_ih, np.float64)
    b_hh = np.asarray(b_hh, np.float64)
    W_head = np.asarray(W_head, np.float64)
    b_head = np.asarray(b_head, np.float64)
    dt = float(dt)

    ONE = H
    ZR = slice(96, 96 + D)

    def fold(A_h, A_z, a0):
        return (
            A_h + dt * (A_z @ W_head),
            A_z,
            a0 + dt * (A_z @ b_head),
        )

    # gate blocks in PyTorch order: r = rows 0:H, u = H:2H, n = 2H:3H
    r_h, r_z, r_0 = fold(W_hh[0:H], W_ih[0:H], b_ih[0:H] + b_hh[0:H])
    u_h, u_z, u_0 = fold(
        W_hh[H : 2 * H], W_ih[H : 2 * H], b_ih[H : 2 * H] + b_hh[H : 2 * H]
    )
    # i_n depends only on z (A_h = 0); h_n only on h (A_z = 0, no fold)
    in_h, in_z, in_0 = fold(np.zeros((H, H)), W_ih[2 * H : 3 * H], b_ih[2 * H :])
    hn_h, hn_0 = W_hh[2 * H : 3 * H], b_hh[2 * H : 3 * H]

    w1 = np.zeros((K, 2 * H))
    # u' block (negated): sigmoid(-a_u) = 1-u
    w1[0:H, 0:H] = -u_h.T
    w1[ZR, 0:H] = -u_z.T
    w1[ONE, 0:H] = -u_0
    # r block
    w1[0:H, H : 2 * H] = r_h.T
    w1[ZR, H : 2 * H] = r_z.T
    w1[ONE, H : 2 * H] = r_0

    w23 = np.zeros((K, 2 * H))
    # i_n block (h-part nonzero after fold!)
    w23[0:H, 0:H] = in_h.T
    w23[ZR, 0:H] = in_z.T
    w23[ONE, 0:H] = in_0
    # h_n block
    w23[0:H, H : 2 * H] = hn_h.T
    w23[ONE, H : 2 * H] = hn_0

    w5 = np.zeros((H + 1, D))
    w5[0:H, :] = dt * W_head.T
    w5[H, :] = dt * b_head

    i1 = np.zeros((H, 2 * H))
    i1[0:H, 0:H] = np.eye(H)
    i2 = np.eye(H)

    # h-seed stationary: all zeros (h0 = 0); z seeding is done on DVE
    zs3 = np.zeros((K, 2 * H))
    bhd = (dt * b_head).reshape(D, 1)

    wz0 = np.zeros((K, 2 * H))

    f32 = np.float32
    return tuple(
        a.astype(f32) for a in (w1, w23, w5, i1, i2, zs3, wz0, bhd)
    ) + (f32(dt), b_head.astype(f32))


def kernel(z0, dt, steps, W_ih, W_hh, b_ih, b_hh, W_head, b_head):
    z0 = np.asarray(z0, np.float32)
    steps = int(steps)
    B, d = z0.shape
    assert (B, d) == (B_FULL, D)
    w1, w23, w5, i1, i2, zs3, wz0, bhd, dtf, bh_f = _pack_weights(
        dt, W_ih, W_hh, b_ih, b_hh, W_head, b_head
    )

    nc = _build(steps)
    in_maps = []
    for c in range(N_CORES):
        z0c = z0[c * BC : (c + 1) * BC]
        xr0 = np.zeros((K, BC), np.float32)
        # lazy-z: mirror seeds with zeta_0 = z0 - dt*b_head
        xr0[96 : 96 + D, :] = (z0c - dtf * bh_f[None, :]).T
        xr0[H, :] = 1.0
        in_maps.append(
            {
                "xr0": xr0,
                "w1": w1,
                "w23": w23,
                "w5": w5,
                "i1": i1,
                "i2": i2,
                "zs3": zs3,
                "wz0": wz0,
                "bhd": bhd,
            }
        )
    res = run_bass_kernel_spmd(nc, in_maps, core_ids=list(range(N_CORES)))

    outs = []
    for c in range(N_CORES):
        zsv = np.asarray(res.results[c]["zs"], np.float32)
        traj = (
            zsv.reshape(steps + 1, D, BC).transpose(2, 0, 1).astype(np.float32)
        )
        outs.append(traj)
    return np.concatenate(outs, axis=0)
